# revision 2
# baseline (speedup 1.0000x reference)
"""Trainium2 Bass kernel for nn_MaxAssigner2D (span=2 shifted channel-max pool).

Math (per image, zero-padded borders):
    m[h, w]   = max_c x[h, w, c]
    out[h, w] = max over (dh, dw) in S of m[h-dh, w-dw]   (0 outside bounds)
    S = {(0,0), (1,0), (0,1), (1,1), (2,0), (0,2), (2,2)}

Distribution: pure data parallel, 2 images per core across 8 NeuronCores.

Per-core layout: partition p <-> 4-row band of the image (128 bands x 4 rows
= 512 rows).  The channel-max plane m for each image lives in SBUF as
[128, 6*514] bf16: per partition 2 halo rows (rows 4p-2, 4p-1, zero for p=0)
followed by the band's 4 rows, each row stored as [2 zero pad cols | 512 data
cols].  With that layout every shift (dh, dw) of the 7-term max is a pure
free-dim AP offset, and the zero padding of the reference comes for free.

v2 changes vs the 231us baseline (DMA 89.8% busy, DVE 72% busy):
  * Input chunk DMAs alternate between the two HWDGE rings (nc.sync /
    nc.scalar).  A single ring serializes the ~2us completion-receipt
    latency per transfer; two rings interleave at SDMA packet granularity
    and hide each other's receipt bubbles.
  * Channel max is a 5-level tensor_max tree instead of reduce_max
    (reduce is a 1x-rate DVE op: 1 elem/cycle).  Level 1 eats f32 at the
    2-read-ports/cycle ceiling and writes bf16 with a transposed
    interleave so levels 2-5 are fully dense bf16 ops (2x DVE rate).
  * m plane, shift accumulator in bf16 (max is order-preserving under
    round-to-nearest, |err| <= 2^-9 rel, tolerance is 2e-2); the store
    DMA casts bf16->f32 on the way out (SWDGE).
  * Halo SBUF->SBUF copy moved off the input rings onto the SWDGE (Pool)
    ring and issued right after its source rows reduce, overlapping the
    remaining input stream.
  * Full-tile memsets replaced by targeted pad zeroing.
"""

import numpy as np

import concourse.bacc as bacc
import concourse.bass as bass
import concourse.mybir as mybir
from concourse.tile import TileContext, add_dep_helper

F32 = mybir.dt.float32
BF16 = mybir.dt.bfloat16
NCORES = 8

# Full-problem geometry (hardcoded; kernel.py must be self-contained).
B, H, W, C = 16, 512, 512, 32
SPAN = 2


def build_nc(bpc, h, w, c, ph, qw):
    """Build the per-core Bass module.

    bpc: images per core; h/w/c: image dims; ph: rows per band (partitions =
    h // ph); qw: pixels per stage-1 chunk per partition.
    """
    P = h // ph               # partitions used
    assert P <= 128
    nq = w // qw              # chunks per band row
    rowp = SPAN + w           # padded row width (left zero pad only)
    mrows = ph + SPAN         # SPAN halo rows + band rows
    msz = mrows * rowp

    # Bacc (not raw Bass): its finalize() runs generate_event_semaphores(),
    # which splits multi-wait instructions to satisfy the TRN2 1-wait limit.
    nc = bacc.Bacc("TRN2")
    x = nc.declare_dram_parameter("x", [bpc, h, w, c], F32, isOutput=False)
    out = nc.declare_dram_parameter("out", [bpc, h, w, 1], F32, isOutput=True)

    # DRAM views: partition p <-> band p
    xr = x.ap().rearrange("b (p ph) w c -> b p ph (w c)", ph=ph)     # [bpc,P,ph,w*c]
    outr = out.ap().rearrange("b (p ph) w c -> b p ph (w c)", ph=ph)  # [bpc,P,ph,w]

    rings = [nc.sync, nc.scalar]   # the two HWDGE rings
    ring_i = [0]

    with TileContext(nc) as tc:
        with (
            tc.tile_pool(name="xp", bufs=4) as xpool,
            tc.tile_pool(name="sp", bufs=3) as spool,
            tc.tile_pool(name="mp", bufs=1) as mpool,
            tc.tile_pool(name="op", bufs=2) as opool,
        ):
            # Persistent per-image m tiles (bf16).
            m_tiles = [
                mpool.tile([P, msz], BF16, tag=f"m{bi}", name=f"m{bi}")
                for bi in range(bpc)
            ]
            # Zero only what stage 3 reads and nothing else writes: the
            # left pad columns of every row, and partition 0's halo rows
            # (the halo DMA only writes partitions 1..P-1).
            for mt in m_tiles:
                mt3 = mt[:].rearrange("p (r w) -> p r w", w=rowp)
                nc.vector.memset(mt3[:, :, 0:SPAN], 0.0)
                nc.gpsimd.memset(mt3[0:1, 0:SPAN, :], 0.0)

            assert nq % 2 == 0
            hw = w // 2

            def chan_max_tree(xt, mt3, r, p0, sw):
                """32ch -> 1 max tree for sw pixels of band row r at col p0.

                L1 (f32 -> bf16, 1x) writes outputs transposed (k-major) so
                L2..L5 read/write flat dense bf16 (2x DVE mode).
                """
                st = spool.tile([P, 30 * sw], BF16, tag="st", name="st")
                x3 = xt[:, 0:sw * c].rearrange("p (w c) -> p w c", c=c)
                # t1[k*sw + w] = max(x[w, k], x[w, 16+k]), k = 0..15
                t1t = st[:, 0:16 * sw].rearrange("p (k w) -> p w k", w=sw)
                nc.vector.tensor_max(t1t, x3[:, :, 0:16], x3[:, :, 16:32])
                o = [0, 16 * sw, 24 * sw, 28 * sw, 30 * sw]
                for l in range(3):  # 16->8, 8->4, 4->2: flat dense bf16
                    n = (16 >> l) * sw // 2
                    nc.vector.tensor_max(
                        st[:, o[l + 1]:o[l + 1] + n],
                        st[:, o[l]:o[l] + n],
                        st[:, o[l] + n:o[l] + 2 * n],
                    )
                # 2->1 straight into the m row
                nc.vector.tensor_max(
                    mt3[:, SPAN + r, SPAN + p0:SPAN + p0 + sw],
                    st[:, o[3]:o[3] + sw],
                    st[:, o[3] + sw:o[3] + 2 * sw],
                )

            for bi in range(bpc):
                mt = m_tiles[bi]
                mt3 = mt[:].rearrange("p (r w) -> p r w", w=rowp)  # [P,mrows,rowp]
                acc = opool.tile([P, ph * w], BF16, tag="acc", name="acc")
                a3 = acc[:].rearrange("p (r w) -> p r w", w=w)  # [P,ph,w]

                # Two w-halves per image: stream a half's chunks, then do its
                # shifts + stores while the next half (or image) streams.
                for half in range(2):
                    w0 = half * hw
                    qs = range(half * nq // 2, (half + 1) * nq // 2)
                    # ---- stage 1: channel max into m tile ----
                    # Halo source rows (ph-2, ph-1) first.  The very first
                    # chunk of the kernel is split into 4 sub-chunks so the
                    # first reduce starts earlier (pipeline fill).
                    first_chunk = bi == 0 and half == 0
                    for ri, r in enumerate((ph - 2, ph - 1, *range(ph - 2))):
                        for q in qs:
                            nsub = 4 if first_chunk else 1
                            first_chunk = False
                            sw = qw // nsub
                            for s in range(nsub):
                                p0 = q * qw + s * sw
                                xt = xpool.tile(
                                    [P, sw * c], F32, tag="xt", name="xt"
                                )
                                ring = rings[ring_i[0] % 2]
                                ring_i[0] += 1
                                ring.dma_start(
                                    out=xt[:, 0:sw * c],
                                    in_=xr[bi, :, r, p0 * c:(p0 + sw) * c],
                                )
                                chan_max_tree(xt, mt3, r, p0, sw)

                        # ---- stage 2: halo rows for this half's columns,
                        # partition-shifted SBUF->SBUF copy on the SWDGE
                        # (Pool) ring, issued as soon as both source rows
                        # are reduced so it overlaps rows 0..ph-3 streaming.
                        if ri == 1:
                            c0 = 0 if half == 0 else SPAN + hw
                            c1 = SPAN + hw if half == 0 else rowp
                            nc.gpsimd.dma_start(
                                out=mt3[1:P, 0:SPAN, c0:c1],
                                in_=mt3[0:P - 1, ph:ph + SPAN, c0:c1],
                            )

                    # ---- stage 3: 7-shift max for this half's outputs ----
                    # Ordered so every op that touches halo rows comes last.
                    def opnd(dh, dw, lo, hi):
                        # operand rows for out band rows [lo, hi)
                        return mt3[
                            :,
                            SPAN + lo - dh:SPAN + hi - dh,
                            SPAN - dw + w0:SPAN - dw + w0 + hw,
                        ]

                    ah = a3[:, :, w0:w0 + hw]

                    def amax(lo, hi, dh, dw, first=False):
                        dst = a3[:, lo:hi, w0:w0 + hw]
                        src0 = opnd(0, 0, lo, hi) if first else dst
                        nc.vector.tensor_max(dst, src0, opnd(dh, dw, lo, hi))

                    # dh=0 terms, all rows (halo-free)
                    amax(0, ph, 0, 1, first=True)
                    amax(0, ph, 0, 2)
                    # dh>=1 terms, inner rows (halo-free)
                    for dh, dw in [(1, 0), (1, 1), (2, 0), (2, 2)]:
                        amax(SPAN, ph, dh, dw)
                    # dh>=1 terms, boundary rows (read halo rows)
                    for dh, dw in [(1, 0), (1, 1), (2, 0), (2, 2)]:
                        amax(0, SPAN, dh, dw)

                    # ---- stage 4: store, casting bf16 -> f32 (SWDGE) ----
                    nc.gpsimd.dma_start(
                        out=outr[bi, :, :, w0:w0 + hw], in_=ah
                    )

    # run_bass_via_pjrt binds the bass_exec primitive without finalizing;
    # Bacc needs finalize() -> compile() for register allocation and the
    # TRN2 one-wait-per-instruction semaphore legalization.
    nc.finalize()
    return nc


_NC_CACHE = {}


def _get_nc():
    key = "full"
    if key not in _NC_CACHE:
        _NC_CACHE[key] = build_nc(B // NCORES, H, W, C, ph=4, qw=256)
    return _NC_CACHE[key]


def _run(x, trace=False):
    """Run the SPMD kernel on 8 cores. Returns (out, BassKernelResults)."""
    from concourse.bass_utils import run_bass_kernel_spmd

    x = np.ascontiguousarray(np.asarray(x), dtype=np.float32)
    assert x.shape == (B, H, W, C)
    bpc = B // NCORES
    nc = _get_nc()
    in_maps = [
        {"x": np.ascontiguousarray(x[i * bpc:(i + 1) * bpc])} for i in range(NCORES)
    ]
    res = run_bass_kernel_spmd(nc, in_maps, list(range(NCORES)), trace=trace)
    out = np.concatenate([res.results[i]["out"] for i in range(NCORES)], axis=0)
    return out, res


def kernel(x):
    out, _ = _run(x, trace=False)
    return out


# revision 4
# speedup vs baseline: 1.5404x; 1.5404x over previous
"""Trainium2 Bass kernel for nn_MaxAssigner2D (span=2 shifted channel-max pool).

Math (per image, zero-padded borders):
    m[h, w]   = max_c x[h, w, c]
    out[h, w] = max over (dh, dw) in S of m[h-dh, w-dw]   (0 outside bounds)
    S = {(0,0), (1,0), (0,1), (1,1), (2,0), (0,2), (2,2)}

Distribution: pure data parallel, 2 images per core across 8 NeuronCores.

Per-core layout: partition p <-> 4-row band of the image (128 bands x 4 rows
= 512 rows).  The channel-max plane m for each image lives in SBUF as
[128, 6*514] bf16: per partition 2 halo rows (rows 4p-2, 4p-1, zero for p=0)
followed by the band's 4 rows, each row stored as [2 zero pad cols | 512 data
cols].  With that layout every shift (dh, dw) of the 7-term max is a pure
free-dim AP offset, and the zero padding of the reference comes for free.

v2 changes vs the 231us baseline (DMA 89.8% busy, DVE 72% busy):
  * Input chunk DMAs alternate between the two HWDGE rings (nc.sync /
    nc.scalar).  A single ring serializes the ~2us completion-receipt
    latency per transfer; two rings interleave at SDMA packet granularity
    and hide each other's receipt bubbles.
  * Channel max is a 5-level tensor_max tree instead of reduce_max
    (reduce is a 1x-rate DVE op: 1 elem/cycle).  Level 1 eats f32 at the
    2-read-ports/cycle ceiling and writes bf16 with a transposed
    interleave so levels 2-5 are fully dense bf16 ops (2x DVE rate).
  * m plane, shift accumulator in bf16 (max is order-preserving under
    round-to-nearest, |err| <= 2^-9 rel, tolerance is 2e-2); the store
    DMA casts bf16->f32 on the way out (SWDGE).
  * Halo SBUF->SBUF copy moved off the input rings onto the SWDGE (Pool)
    ring and issued right after its source rows reduce, overlapping the
    remaining input stream.
  * Full-tile memsets replaced by targeted pad zeroing.
"""

import numpy as np

import concourse.bacc as bacc
import concourse.bass as bass
import concourse.mybir as mybir
from concourse.tile import TileContext, add_dep_helper

F32 = mybir.dt.float32
BF16 = mybir.dt.bfloat16
NCORES = 8

# Full-problem geometry (hardcoded; kernel.py must be self-contained).
B, H, W, C = 16, 512, 512, 32
SPAN = 2


def build_nc(bpc, h, w, c, ph, qw):
    """Build the per-core Bass module.

    bpc: images per core; h/w/c: image dims; ph: rows per band (partitions =
    h // ph); qw: pixels per stage-1 chunk per partition.
    """
    P = h // ph               # partitions used
    assert P <= 128
    nq = w // qw              # chunks per band row
    rowp = SPAN + w           # padded row width (left zero pad only)
    mrows = ph + SPAN         # SPAN halo rows + band rows
    msz = mrows * rowp

    # Bacc (not raw Bass): its finalize() runs generate_event_semaphores(),
    # which splits multi-wait instructions to satisfy the TRN2 1-wait limit.
    nc = bacc.Bacc("TRN2")
    x = nc.declare_dram_parameter("x", [bpc, h, w, c], F32, isOutput=False)
    out = nc.declare_dram_parameter("out", [bpc, h, w, 1], F32, isOutput=True)

    # DRAM views: partition p <-> band p
    xr = x.ap().rearrange("b (p ph) w c -> b p ph (w c)", ph=ph)     # [bpc,P,ph,w*c]
    outr = out.ap().rearrange("b (p ph) w c -> b p ph (w c)", ph=ph)  # [bpc,P,ph,w]

    rings = [nc.sync, nc.scalar]   # the two HWDGE rings
    ring_i = [0]

    with TileContext(nc) as tc:
        with (
            tc.tile_pool(name="xp", bufs=4) as xpool,
            tc.tile_pool(name="sp", bufs=3) as spool,
            tc.tile_pool(name="mp", bufs=1) as mpool,
            tc.tile_pool(name="op", bufs=2) as opool,
        ):
            # Persistent per-image m tiles (bf16).
            m_tiles = [
                mpool.tile([P, msz], BF16, tag=f"m{bi}", name=f"m{bi}")
                for bi in range(bpc)
            ]
            # Zero only what stage 3 reads and nothing else writes: the
            # left pad columns of every row, and partition 0's halo rows
            # (the halo DMA only writes partitions 1..P-1).
            for mt in m_tiles:
                mt3 = mt[:].rearrange("p (r w) -> p r w", w=rowp)
                nc.vector.memset(mt3[:, :, 0:SPAN], 0.0)
                nc.gpsimd.memset(mt3[0:1, 0:SPAN, :], 0.0)

            assert nq % 2 == 0
            hw = w // 2

            def chan_max_tree(xt, mt3, r, p0, sw):
                """32ch -> 1 max tree for sw pixels of band row r at col p0.

                Every level is pixel-major with the k (channel) axis
                innermost, so ALL operands walk contiguous runs (16, 8, 4,
                2 elems).  A strided inner dim measures ~4.3 cyc/elem on
                DVE (the v2 regression); contiguous runs go at 1 cyc/elem
                f32 and 0.5-0.7 cyc/elem for dense bf16 (2x mode).
                """
                st = spool.tile([P, 30 * sw], BF16, tag="st", name="st")
                x3 = xt[:, 0:sw * c].rearrange("p (w c) -> p w c", c=c)
                o = [0, 16 * sw, 24 * sw, 28 * sw, 30 * sw]
                # L1: t1[w, k] = max(x[w, k], x[w, 16+k])  (f32 -> bf16)
                t1 = st[:, o[0]:o[1]].rearrange("p (w k) -> p w k", k=16)
                nc.vector.tensor_max(t1, x3[:, :, 0:16], x3[:, :, 16:32])
                for l in range(3):  # 16->8, 8->4, 4->2, all pixel-major
                    ks = 16 >> l
                    src = st[:, o[l]:o[l + 1]].rearrange(
                        "p (w k) -> p w k", k=ks
                    )
                    dst = st[:, o[l + 1]:o[l + 2]].rearrange(
                        "p (w k) -> p w k", k=ks // 2
                    )
                    nc.vector.tensor_max(
                        dst, src[:, :, 0:ks // 2], src[:, :, ks // 2:ks]
                    )
                # 2->1 straight into the m row (stride-2 pair reads)
                nc.vector.tensor_max(
                    mt3[:, SPAN + r, SPAN + p0:SPAN + p0 + sw],
                    st[:, o[3]:o[4]:2],
                    st[:, o[3] + 1:o[4]:2],
                )

            for bi in range(bpc):
                mt = m_tiles[bi]
                mt3 = mt[:].rearrange("p (r w) -> p r w", w=rowp)  # [P,mrows,rowp]
                acc = opool.tile([P, ph * w], BF16, tag="acc", name="acc")
                a3 = acc[:].rearrange("p (r w) -> p r w", w=w)  # [P,ph,w]

                # Two w-halves per image: stream a half's chunks, then do its
                # shifts + stores while the next half (or image) streams.
                for half in range(2):
                    w0 = half * hw
                    qs = range(half * nq // 2, (half + 1) * nq // 2)
                    # ---- stage 1: channel max into m tile ----
                    # Halo source rows (ph-2, ph-1) first.  The very first
                    # chunk of the kernel is split into 4 sub-chunks so the
                    # first reduce starts earlier (pipeline fill).
                    first_chunk = bi == 0 and half == 0
                    last_half = bi == bpc - 1 and half == 1
                    rows = (ph - 2, ph - 1, *range(ph - 2))
                    for ri, r in enumerate(rows):
                        for q in qs:
                            # Split the first chunk (pipeline fill) and the
                            # last chunk (drain tail) into 4 sub-chunks.
                            last_chunk = (
                                last_half and ri == len(rows) - 1 and q == qs[-1]
                            )
                            nsub = 4 if (first_chunk or last_chunk) else 1
                            first_chunk = False
                            sw = qw // nsub
                            for s in range(nsub):
                                p0 = q * qw + s * sw
                                xt = xpool.tile(
                                    [P, sw * c], F32, tag="xt", name="xt"
                                )
                                ring = rings[ring_i[0] % 2]
                                ring_i[0] += 1
                                ring.dma_start(
                                    out=xt[:, 0:sw * c],
                                    in_=xr[bi, :, r, p0 * c:(p0 + sw) * c],
                                )
                                chan_max_tree(xt, mt3, r, p0, sw)

                        # ---- stage 2: halo rows for this half's columns,
                        # partition-shifted SBUF->SBUF copy on the SWDGE
                        # (Pool) ring, issued as soon as both source rows
                        # are reduced so it overlaps rows 0..ph-3 streaming.
                        if ri == 1:
                            c0 = 0 if half == 0 else SPAN + hw
                            c1 = SPAN + hw if half == 0 else rowp
                            nc.gpsimd.dma_start(
                                out=mt3[1:P, 0:SPAN, c0:c1],
                                in_=mt3[0:P - 1, ph:ph + SPAN, c0:c1],
                            )

                    # ---- stage 3: 7-shift max for this half's outputs ----
                    # Ordered so every op that touches halo rows comes last.
                    def opnd(dh, dw, lo, hi):
                        # operand rows for out band rows [lo, hi)
                        return mt3[
                            :,
                            SPAN + lo - dh:SPAN + hi - dh,
                            SPAN - dw + w0:SPAN - dw + w0 + hw,
                        ]

                    ah = a3[:, :, w0:w0 + hw]

                    def amax(lo, hi, dh, dw, first=False):
                        dst = a3[:, lo:hi, w0:w0 + hw]
                        src0 = opnd(0, 0, lo, hi) if first else dst
                        nc.vector.tensor_max(dst, src0, opnd(dh, dw, lo, hi))

                    # dh=0 terms, all rows (halo-free)
                    amax(0, ph, 0, 1, first=True)
                    amax(0, ph, 0, 2)
                    # dh>=1 terms, inner rows (halo-free)
                    for dh, dw in [(1, 0), (1, 1), (2, 0), (2, 2)]:
                        amax(SPAN, ph, dh, dw)
                    # dh>=1 terms, boundary rows (read halo rows)
                    for dh, dw in [(1, 0), (1, 1), (2, 0), (2, 2)]:
                        amax(0, SPAN, dh, dw)

                    # ---- stage 4: store, casting bf16 -> f32 (SWDGE) ----
                    nc.gpsimd.dma_start(
                        out=outr[bi, :, :, w0:w0 + hw], in_=ah
                    )

    # run_bass_via_pjrt binds the bass_exec primitive without finalizing;
    # Bacc needs finalize() -> compile() for register allocation and the
    # TRN2 one-wait-per-instruction semaphore legalization.
    nc.finalize()
    return nc


_NC_CACHE = {}


def _get_nc():
    key = "full"
    if key not in _NC_CACHE:
        _NC_CACHE[key] = build_nc(B // NCORES, H, W, C, ph=4, qw=256)
    return _NC_CACHE[key]


def _run(x, trace=False):
    """Run the SPMD kernel on 8 cores. Returns (out, BassKernelResults)."""
    from concourse.bass_utils import run_bass_kernel_spmd

    x = np.ascontiguousarray(np.asarray(x), dtype=np.float32)
    assert x.shape == (B, H, W, C)
    bpc = B // NCORES
    nc = _get_nc()
    in_maps = [
        {"x": np.ascontiguousarray(x[i * bpc:(i + 1) * bpc])} for i in range(NCORES)
    ]
    res = run_bass_kernel_spmd(nc, in_maps, list(range(NCORES)), trace=trace)
    out = np.concatenate([res.results[i]["out"] for i in range(NCORES)], axis=0)
    return out, res


def kernel(x):
    out, _ = _run(x, trace=False)
    return out


# revision 6
# speedup vs baseline: 1.8291x; 1.1874x over previous
"""Trainium2 Bass kernel for nn_MaxAssigner2D (span=2 shifted channel-max pool).

Math (per image, zero-padded borders):
    m[h, w]   = max_c x[h, w, c]
    out[h, w] = max over (dh, dw) in S of m[h-dh, w-dw]   (0 outside bounds)
    S = {(0,0), (1,0), (0,1), (1,1), (2,0), (0,2), (2,2)}

Distribution: pure data parallel, 2 images per core across 8 NeuronCores.

Per-core layout: partition p <-> 4-row band of the image (128 bands x 4 rows
= 512 rows).  The channel-max plane m for each image lives in SBUF as
[128, 6*514] bf16: per partition 2 halo rows (rows 4p-2, 4p-1, zero for p=0)
followed by the band's 4 rows, each row stored as [2 zero pad cols | 512 data
cols].  With that layout every shift (dh, dw) of the 7-term max is a pure
free-dim AP offset, and the zero padding of the reference comes for free.

Engine plan (from v1-v3 traces):
  * Input stream: 4 MiB chunk DMAs alternating between the two HWDGE rings
    (nc.sync / nc.scalar).  One ring serializes its ~2us completion-receipt
    latency per transfer; two rings interleave at SDMA packet granularity
    and hide each other's bubbles (~425 GB/s sustained vs 323 single-ring).
  * Channel max: 5-level DVE tensor_max tree, pixel-major so every level
    walks contiguous runs (a strided inner dim costs ~4.3 cyc/elem vs 1).
    L1 eats f32 at the 2-read-port ceiling and writes bf16; L2-5 run in
    bf16 2x mode.  DVE total ~129us vs ~170us for the 1x reduce_max.
  * m plane, accumulator in bf16 (max is order-preserving under round-to-
    nearest, |err| <= 2^-9 rel, tolerance 2e-2); the store DMA casts
    bf16 -> f32 on the way out (SWDGE, overlapped).
  * Halo: partition-shift via the idle TensorEngine (shifted-identity
    matmul into PSUM, ACT engine copies back to SBUF).  Halo-as-DMA is
    poison: its ~254 tiny descriptors pile onto ONE SDMA engine which
    round-robins them against the fat input stream - 20-50us completion,
    and the boundary shift-max ops stall on it (the v3 83->113us DVE gap).
  * First/last chunk are split into 4 sub-DMAs into ONE xt tile (fast
    pipeline fill/drain without eating 4 xpool buffer slots - the v3
    startup took 35us to reach steady state because the subs strangled
    the prefetch depth).
"""

import numpy as np

import concourse.bacc as bacc
import concourse.bass as bass
import concourse.mybir as mybir
from concourse.tile import TileContext, add_dep_helper

F32 = mybir.dt.float32
BF16 = mybir.dt.bfloat16
NCORES = 8

# Full-problem geometry (hardcoded; kernel.py must be self-contained).
B, H, W, C = 16, 512, 512, 32
SPAN = 2


def build_nc(bpc, h, w, c, ph, qw):
    """Build the per-core Bass module.

    bpc: images per core; h/w/c: image dims; ph: rows per band (partitions =
    h // ph); qw: pixels per stage-1 chunk per partition.
    """
    P = h // ph               # partitions used
    assert P <= 128
    nq = w // qw              # chunks per band row
    rowp = SPAN + w           # padded row width (left zero pad only)
    mrows = ph + SPAN         # SPAN halo rows + band rows
    msz = mrows * rowp

    # Bacc (not raw Bass): its finalize() runs generate_event_semaphores(),
    # which splits multi-wait instructions to satisfy the TRN2 1-wait limit.
    nc = bacc.Bacc("TRN2")
    x = nc.declare_dram_parameter("x", [bpc, h, w, c], F32, isOutput=False)
    out = nc.declare_dram_parameter("out", [bpc, h, w, 1], F32, isOutput=True)

    # DRAM views: partition p <-> band p
    xr = x.ap().rearrange("b (p ph) w c -> b p ph (w c)", ph=ph)     # [bpc,P,ph,w*c]
    outr = out.ap().rearrange("b (p ph) w c -> b p ph (w c)", ph=ph)  # [bpc,P,ph,w]

    rings = [nc.sync, nc.scalar]   # the two HWDGE rings
    ring_i = [0]

    with TileContext(nc) as tc:
        with (
            tc.tile_pool(name="xp", bufs=4) as xpool,
            tc.tile_pool(name="sp", bufs=2) as spool,
            tc.tile_pool(name="mp", bufs=1) as mpool,
            tc.tile_pool(name="op", bufs=4) as opool,
            tc.psum_pool(name="pp", bufs=2) as ppool,
        ):
            # Shifted identity for the TensorE halo: S[q, p] = 1 iff q = p-1,
            # so (S.T @ m)[p, :] = m[p-1, :] (and row p=0 = 0, the zero pad).
            ident = mpool.tile([P, P], BF16, tag="ident", name="ident")
            nc.gpsimd.memset(ident[:], 0.0)
            nc.gpsimd.affine_select(
                out=ident[:],
                in_=ident[:],
                compare_op=mybir.AluOpType.not_equal,
                fill=1.0,
                base=1,
                # iota[q, p] = 1 + q - p; == 0 exactly at p = q + 1
                pattern=[[-1, P]],
                channel_multiplier=1,
            )

            # Persistent per-image m tiles (bf16).
            m_tiles = [
                mpool.tile([P, msz], BF16, tag=f"m{bi}", name=f"m{bi}")
                for bi in range(bpc)
            ]
            # Zero the left pad columns of every row once (the halo copy
            # refreshes the halo rows incl. their pads each image/half).
            for mt in m_tiles:
                mt3 = mt[:].rearrange("p (r w) -> p r w", w=rowp)
                nc.vector.memset(mt3[:, :, 0:SPAN], 0.0)

            assert nq % 2 == 0
            hw = w // 2

            def chan_max_tree(xt, x0, mt3, r, p0, sw):
                """32ch -> 1 max tree for sw pixels of band row r at col p0.

                Reads xt[:, x0 : x0 + sw*c].  Every level is pixel-major with
                the k (channel) axis innermost, so ALL operands walk
                contiguous runs (16, 8, 4, 2 elems).  A strided inner dim
                measures ~4.3 cyc/elem on DVE (the v2 regression);
                contiguous runs go at 1 cyc/elem f32 and 0.5-0.7 cyc/elem
                dense bf16 (2x mode).
                """
                st = spool.tile([P, 30 * sw], BF16, tag="st", name="st")
                x3 = xt[:, x0:x0 + sw * c].rearrange("p (w c) -> p w c", c=c)
                o = [0, 16 * sw, 24 * sw, 28 * sw, 30 * sw]
                # L1: t1[w, k] = max(x[w, k], x[w, 16+k])  (f32 -> bf16)
                t1 = st[:, o[0]:o[1]].rearrange("p (w k) -> p w k", k=16)
                nc.vector.tensor_max(t1, x3[:, :, 0:16], x3[:, :, 16:32])
                for l in range(3):  # 16->8, 8->4, 4->2, all pixel-major
                    ks = 16 >> l
                    src = st[:, o[l]:o[l + 1]].rearrange(
                        "p (w k) -> p w k", k=ks
                    )
                    dst = st[:, o[l + 1]:o[l + 2]].rearrange(
                        "p (w k) -> p w k", k=ks // 2
                    )
                    nc.vector.tensor_max(
                        dst, src[:, :, 0:ks // 2], src[:, :, ks // 2:ks]
                    )
                # 2->1 straight into the m row (stride-2 pair reads)
                nc.vector.tensor_max(
                    mt3[:, SPAN + r, SPAN + p0:SPAN + p0 + sw],
                    st[:, o[3]:o[4]:2],
                    st[:, o[3] + 1:o[4]:2],
                )

            for bi in range(bpc):
                mt = m_tiles[bi]
                mt3 = mt[:].rearrange("p (r w) -> p r w", w=rowp)  # [P,mrows,rowp]
                acc = opool.tile([P, ph * w], BF16, tag="acc", name="acc")
                a3 = acc[:].rearrange("p (r w) -> p r w", w=w)  # [P,ph,w]

                # Two w-halves per image: stream a half's chunks, then do its
                # shifts + stores while the next half (or image) streams.
                for half in range(2):
                    w0 = half * hw
                    qs = range(half * nq // 2, (half + 1) * nq // 2)
                    # ---- stage 1: channel max into m tile ----
                    # Halo source rows (ph-2, ph-1) stream first.
                    first_chunk = bi == 0 and half == 0
                    last_half = bi == bpc - 1 and half == 1
                    rows = (ph - 2, ph - 1, *range(ph - 2))
                    for ri, r in enumerate(rows):
                        for q in qs:
                            # Split the first chunk (pipeline fill) and the
                            # last chunk (drain tail) into 4 sub-DMAs into a
                            # single xt tile (keeps prefetch depth).
                            last_chunk = (
                                last_half and ri == len(rows) - 1 and q == qs[-1]
                            )
                            nsub = 4 if (first_chunk or last_chunk) else 1
                            first_chunk = False
                            sw = qw // nsub
                            xt = xpool.tile([P, qw * c], F32, tag="xt", name="xt")
                            for s in range(nsub):
                                p0 = q * qw + s * sw
                                ring = rings[ring_i[0] % 2]
                                ring_i[0] += 1
                                ring.dma_start(
                                    out=xt[:, s * sw * c:(s + 1) * sw * c],
                                    in_=xr[bi, :, r, p0 * c:(p0 + sw) * c],
                                )
                                chan_max_tree(xt, s * sw * c, mt3, r, p0, sw)

                        # ---- stage 2: halo rows for this half's columns.
                        # TensorE partition shift: psum[p] = m_row[p-1], one
                        # matmul per source row (moving free dim <= 512),
                        # then ACT copies PSUM -> the two halo rows.
                        if ri == 1:
                            c0 = 0 if half == 0 else SPAN + hw
                            c1 = SPAN + hw if half == 0 else rowp
                            L = c1 - c0
                            # One full PSUM bank (512 f32) per source row: a
                            # matmul output must not straddle a bank boundary
                            # (straddling corrupted the seam columns in v4).
                            BANK = 512
                            pt = ppool.tile(
                                [P, SPAN * BANK], F32, tag="halo", name="halo"
                            )
                            pt3 = pt[:].rearrange("p (j l) -> p j l", l=BANK)
                            for j in range(SPAN):
                                nc.tensor.matmul(
                                    pt3[:, j, 0:L],
                                    ident[:],
                                    mt3[:, ph + j, c0:c1],
                                    start=True,
                                    stop=True,
                                )
                            nc.scalar.copy(
                                out=mt3[:, 0:SPAN, c0:c1],
                                in_=pt3[:, :, 0:L],
                            )

                    # ---- stage 3: 7-shift max for this half's outputs ----
                    # Ordered so every op that touches halo rows comes last.
                    def opnd(dh, dw, lo, hi):
                        # operand rows for out band rows [lo, hi)
                        return mt3[
                            :,
                            SPAN + lo - dh:SPAN + hi - dh,
                            SPAN - dw + w0:SPAN - dw + w0 + hw,
                        ]

                    ah = a3[:, :, w0:w0 + hw]

                    def amax(lo, hi, dh, dw, first=False):
                        dst = a3[:, lo:hi, w0:w0 + hw]
                        src0 = opnd(0, 0, lo, hi) if first else dst
                        nc.vector.tensor_max(dst, src0, opnd(dh, dw, lo, hi))

                    # dh=0 terms, all rows (halo-free)
                    amax(0, ph, 0, 1, first=True)
                    amax(0, ph, 0, 2)
                    # dh>=1 terms, inner rows (halo-free)
                    for dh, dw in [(1, 0), (1, 1), (2, 0), (2, 2)]:
                        amax(SPAN, ph, dh, dw)
                    # dh>=1 terms, boundary rows (read halo rows)
                    for dh, dw in [(1, 0), (1, 1), (2, 0), (2, 2)]:
                        amax(0, SPAN, dh, dw)

                    # ---- stage 4: store, casting bf16 -> f32 (SWDGE) ----
                    nc.gpsimd.dma_start(
                        out=outr[bi, :, :, w0:w0 + hw], in_=ah
                    )

    # run_bass_via_pjrt binds the bass_exec primitive without finalizing;
    # Bacc needs finalize() -> compile() for register allocation and the
    # TRN2 one-wait-per-instruction semaphore legalization.
    nc.finalize()
    return nc


_NC_CACHE = {}


def _get_nc():
    key = "full"
    if key not in _NC_CACHE:
        _NC_CACHE[key] = build_nc(B // NCORES, H, W, C, ph=4, qw=256)
    return _NC_CACHE[key]


def _run(x, trace=False):
    """Run the SPMD kernel on 8 cores. Returns (out, BassKernelResults)."""
    from concourse.bass_utils import run_bass_kernel_spmd

    x = np.ascontiguousarray(np.asarray(x), dtype=np.float32)
    assert x.shape == (B, H, W, C)
    bpc = B // NCORES
    nc = _get_nc()
    in_maps = [
        {"x": np.ascontiguousarray(x[i * bpc:(i + 1) * bpc])} for i in range(NCORES)
    ]
    res = run_bass_kernel_spmd(nc, in_maps, list(range(NCORES)), trace=trace)
    out = np.concatenate([res.results[i]["out"] for i in range(NCORES)], axis=0)
    return out, res


def kernel(x):
    out, _ = _run(x, trace=False)
    return out
